# revision 1
# baseline (speedup 1.0000x reference)
# Trainium2 Bass kernel for CapsNet dynamic routing (nn_DigiCap).
#
#   u_hat = einsum('bid,ikdo->biko', x, W);  3 routing iterations of
#   softmax / weighted-sum / squash over K=32 output capsules.
#
# Strategy: shard the 2048 input capsules (i) across 8 cores (I_loc=256).
# Each core:
#   Phase A: compute u_hat for its i-slice on the PE using 32x64 array
#     tiling (8 concurrent sub-tiles, one input capsule each), evacuate
#     PSUM->SBUF as bf16, and accumulate s0 = sum_i u_hat in PSUM via a
#     second matmul per tile.
#   Passes t=1,2: b_t = sum_o u_hat*G_t (G_t = v0+..+v_{t-1}),
#     c = softmax_k(b), s_t(partial) = sum_i c*u_hat  -- all on DVE with
#     bf16 2x tensor_tensor ops and binary-tree reductions.
#   Partial s is all-reduced across cores (DRAM bounce + AllReduce),
#   squash computed redundantly per core.
#
# SBUF layout: u_hat [128 part = (i_hi*64+b), i_lo=128, o=16, k=32] bf16.

import numpy as np

B, I_TOT, D, K, O = 64, 2048, 16, 32, 16
NCORES = 8
I_LOC = I_TOT // NCORES     # 256
NJ = 32                     # bursts of 8 input capsules
IC = 16                     # i_lo chunk for routing passes

_CACHE = {}


def _build_bass():
    import concourse.bacc as bacc
    import concourse.tile as tile
    from concourse import mybir

    f32 = mybir.dt.float32
    bf16 = mybir.dt.bfloat16
    Alu = mybir.AluOpType
    Act = mybir.ActivationFunctionType
    X = mybir.AxisListType.X

    nc = bacc.Bacc("TRN2", target_bir_lowering=False, debug=False,
                   num_devices=NCORES)

    xT_d = nc.dram_tensor("xT", [128, 64, 64], f32, kind="ExternalInput")
    W_d = nc.dram_tensor("Wre", [NJ, 128, 2, 512], f32, kind="ExternalInput")
    v_d = nc.dram_tensor("v_out", [64, 512], f32, kind="ExternalOutput")

    with tile.TileContext(nc) as tc:
        with (
            tc.tile_pool(name="big", bufs=1) as big,
            tc.tile_pool(name="psum", bufs=1, space="PSUM") as psp,
            tc.tile_pool(name="dram", bufs=1, space="DRAM") as dp,
        ):
            u = big.tile([128, 128, 16, 32], bf16)       # 128 KB/part
            bA = big.tile([128, 128, 32], f32)           # 16 KB/part

            pu = psp.tile([128, 4, 512], f32)            # banks 0-3
            ps0 = psp.tile([128, 4, 512], f32)           # banks 4-7

            # ---------------- Phase A: u_hat + s0 ----------------
            with (
                tc.tile_pool(name="pha", bufs=1) as pha,
                tc.tile_pool(name="wp", bufs=3) as wp,
            ):
                xT = pha.tile([128, 64, 64], f32)
                nc.sync.dma_start(xT[:], xT_d.ap())
                for j in range(NJ):
                    w = wp.tile([128, 2, 512], f32, tag="wburst")
                    nc.sync.dma_start(w[:], W_d[j])
                    for r in range(4):
                        for cg in range(2):
                            m = 32 * cg + j
                            lhsT = xT[32 * r:32 * r + 16, m, :]  # [16, 64]
                            rhs = w[32 * r:32 * r + 16, cg, :]   # [16, 512]
                            nc.tensor.matmul(
                                pu[64 * cg:64 * cg + 64, r, :], lhsT, rhs,
                                start=True, stop=True,
                                tile_position=(32 * r, 64 * cg))
                            nc.tensor.matmul(
                                ps0[64 * cg:64 * cg + 64, r, :], lhsT, rhs,
                                start=(j == 0), stop=(j == NJ - 1),
                                tile_position=(32 * r, 64 * cg))
                    # evacuate: pu free (r, k*16+o) -> u free (i_lo, o, k)
                    src = pu[:].rearrange("p r (k o) -> p r o k", k=K)
                    nc.vector.tensor_copy(u[:, 4 * j:4 * j + 2], src[:, 0:2])
                    nc.scalar.copy(u[:, 4 * j + 2:4 * j + 4], src[:, 2:4])

            with (
                tc.tile_pool(name="small", bufs=1) as sp,
                tc.tile_pool(name="work", bufs=1) as wk,
            ):
                # s0 = (1/K) * sum over the 4 r-banks (c0 uniform = 1/K)
                ps0v = ps0[:].rearrange("p r (k o) -> p r o k", k=K)
                t01 = wk.tile([128, 16, 32], f32, tag="t01")
                s_p = wk.tile([128, 16, 32], f32, tag="s_p")
                nc.vector.tensor_copy(t01[:], ps0v[:, 0])
                nc.vector.tensor_add(t01[:], t01[:], ps0v[:, 1])
                nc.vector.tensor_add(t01[:], t01[:], ps0v[:, 2])
                nc.vector.tensor_add(s_p[:], t01[:], ps0v[:, 3])
                nc.vector.tensor_scalar_mul(s_p[:], s_p[:], 1.0 / K)

                # ------------- shared helpers (emitted inline) -------------
                bounce = dp.tile([64, 512], f32)
                bounce2 = dp.tile([64, 512], f32)

                def allreduce_s(s_tile):
                    # fold i_hi halves (partitions 64-127 into 0-63) during the
                    # DMA into the DRAM bounce buffer, then AllReduce.
                    flat_lo = s_tile[0:64].rearrange("p o k -> p (o k)")
                    flat_hi = s_tile[64:128].rearrange("p o k -> p (o k)")
                    nc.gpsimd.dma_start(bounce[:], flat_lo)
                    nc.gpsimd.dma_start(bounce[:], flat_hi, accum_op=Alu.add)
                    nc.gpsimd.collective_compute(
                        "AllReduce", Alu.add,
                        replica_groups=[list(range(NCORES))],
                        ins=[bounce.opt()], outs=[bounce2.opt()])
                    s_full = wk.tile([64, 16, 32], f32, tag="s_full")
                    nc.sync.dma_start(
                        s_full[:].rearrange("p o k -> p (o k)"), bounce2[:])
                    return s_full

                def squash(s_full, vout):
                    # vout = s * sqrt(n2)/(1+n2),  n2 = sum_o s^2  per (b,k)
                    ss = wk.tile([64, 16, 32], f32, tag="ss")
                    nc.vector.tensor_mul(ss[:], s_full[:], s_full[:])
                    n2 = wk.tile([64, 32], f32, tag="n2")
                    nc.vector.tensor_reduce(
                        n2[:], ss[:].rearrange("p o k -> p k o"), axis=X,
                        op=Alu.add)
                    lnt = wk.tile([64, 32], f32, tag="lnt")
                    nc.scalar.activation(lnt[:], n2[:], Act.Ln)
                    sq = wk.tile([64, 32], f32, tag="sq")
                    nc.scalar.activation(sq[:], lnt[:], Act.Exp, scale=0.5)
                    den = wk.tile([64, 32], f32, tag="den")
                    nc.vector.tensor_scalar_add(den[:], n2[:], 1.0)
                    rec = wk.tile([64, 32], f32, tag="rec")
                    nc.vector.reciprocal(rec[:], den[:])
                    scl = wk.tile([64, 32], f32, tag="scl")
                    nc.vector.tensor_mul(scl[:], sq[:], rec[:])
                    sclb = scl[:].unsqueeze(1).broadcast_to([64, 16, 32])
                    nc.vector.tensor_mul(vout[:], s_full[:], sclb)

                G = wk.tile([64, 16, 32], f32, tag="G")
                vt = wk.tile([64, 16, 32], f32, tag="vt")
                Grep = sp.tile([128, 16, 32], bf16)

                def update_g(first):
                    if first:
                        nc.vector.tensor_copy(G[:], vt[:])
                    else:
                        nc.vector.tensor_add(G[:], G[:], vt[:])
                    # replicate to both partition halves, f32 -> bf16 cast
                    nc.gpsimd.dma_start(Grep[0:64], G[:])
                    nc.gpsimd.dma_start(Grep[64:128], G[:])

                # ---------------- v0 ----------------
                s_full = allreduce_s(s_p)
                squash(s_full, vt)
                update_g(first=True)

                # ---------------- routing passes ----------------
                cT = sp.tile([128, 128, 32], bf16)
                tmp = wk.tile([128, IC, 16, 32], bf16, tag="tmp")
                tl = wk.tile([128, 16, 32], f32, tag="tl")

                for t in (1, 2):
                    # b = sum_o u * G
                    for i0 in range(0, 128, IC):
                        gb = Grep[:].unsqueeze(1).broadcast_to([128, IC, 16, 32])
                        nc.vector.tensor_mul(tmp[:], u[:, i0:i0 + IC], gb)
                        nc.vector.tensor_add(
                            tmp[:, :, 0:8], tmp[:, :, 0:8], tmp[:, :, 8:16])
                        nc.vector.tensor_add(
                            tmp[:, :, 0:4], tmp[:, :, 0:4], tmp[:, :, 4:8])
                        nc.vector.tensor_add(
                            tmp[:, :, 0:2], tmp[:, :, 0:2], tmp[:, :, 2:4])
                        nc.vector.tensor_add(
                            bA[:, i0:i0 + IC, :], tmp[:, :, 0], tmp[:, :, 1])
                    # c = softmax_k(b)   (no max subtraction; |b| is small)
                    nc.scalar.activation(bA[:], bA[:], Act.Exp)
                    Z = wk.tile([128, 128], f32, tag="Z")
                    nc.vector.tensor_reduce(Z[:], bA[:], axis=X, op=Alu.add)
                    rz = wk.tile([128, 128], f32, tag="rz")
                    nc.vector.reciprocal(rz[:], Z[:])
                    rzb = rz[:].unsqueeze(2).broadcast_to([128, 128, 32])
                    nc.vector.tensor_mul(cT[:], bA[:], rzb)
                    # s(partial) = sum_i c * u
                    for i0 in range(0, 128, IC):
                        cb = cT[:, i0:i0 + IC, :].unsqueeze(2).broadcast_to(
                            [128, IC, 16, 32])
                        nc.vector.tensor_mul(tmp[:], u[:, i0:i0 + IC], cb)
                        nc.vector.tensor_add(
                            tmp[0:128, 0:8], tmp[0:128, 0:8], tmp[0:128, 8:16])
                        nc.vector.tensor_add(
                            tmp[0:128, 0:4], tmp[0:128, 0:4], tmp[0:128, 4:8])
                        nc.vector.tensor_add(
                            tmp[0:128, 0:2], tmp[0:128, 0:2], tmp[0:128, 2:4])
                        if i0 == 0:
                            nc.vector.tensor_add(s_p[:], tmp[:, 0], tmp[:, 1])
                        else:
                            nc.vector.tensor_add(tl[:], tmp[:, 0], tmp[:, 1])
                            nc.vector.tensor_add(s_p[:], s_p[:], tl[:])
                    s_full = allreduce_s(s_p)
                    squash(s_full, vt)
                    if t == 1:
                        update_g(first=False)

                # write out v2 (free layout o*32+k; host reorders)
                nc.sync.dma_start(
                    v_d.ap(), vt[:].rearrange("p o k -> p (o k)"))

    nc.compile()
    return nc


def _prep_inputs(x, W):
    """Host-side shard + relayout. Returns per-core input maps."""
    in_maps = []
    for c in range(NCORES):
        lo = c * I_LOC
        xc = x[:, lo:lo + I_LOC, :]                      # [64, 256, 16]
        # xT[32*(i%4)+d, i//4, b] = xc[b, i, d]
        xr = xc.reshape(B, 64, 4, D).transpose(2, 3, 1, 0)   # [4, 16, 64, 64]
        xT = np.zeros((4, 32, 64, 64), dtype=np.float32)
        xT[:, :16] = xr
        xT = np.ascontiguousarray(xT.reshape(128, 64, 64))
        # Wre[j, 32r+d, cg, k*16+o] = W[lo + 128cg + 4j + r, k, d, o]
        Wc = W[lo:lo + I_LOC]                            # [256, 32, 16, 16]
        Wr = Wc.reshape(2, NJ, 4, K, D, O).transpose(1, 2, 4, 0, 3, 5)
        # -> [j, r, d, cg, k, o]
        Wre = np.zeros((NJ, 4, 32, 2, K, O), dtype=np.float32)
        Wre[:, :, :16] = Wr
        Wre = np.ascontiguousarray(Wre.reshape(NJ, 128, 2, K * O))
        in_maps.append({"xT": xT, "Wre": Wre})
    return in_maps


def kernel(**inputs):
    from concourse.bass_utils import run_bass_kernel_spmd

    x = np.ascontiguousarray(inputs["inputs"], dtype=np.float32)
    W = np.ascontiguousarray(inputs["W"], dtype=np.float32)

    if "nc" not in _CACHE:
        _CACHE["nc"] = _build_bass()
    nc = _CACHE["nc"]

    in_maps = _prep_inputs(x, W)
    res = run_bass_kernel_spmd(nc, in_maps, core_ids=list(range(NCORES)))
    v = res.results[0]["v_out"]                          # [64, 512] (o,k)
    return np.ascontiguousarray(
        v.reshape(B, O, K).transpose(0, 2, 1)).astype(np.float32)

